# revision 30
# baseline (speedup 1.0000x reference)
"""Distributed sliding-window GQA attention kernel for 8 Trainium2 NeuronCores.

Problem (full shapes): x [1, 2048, 4096] f32, wq [4096, 4096], wk/wv [4096, 1024],
wo [4096, 4096], rotary freqs [2048, 64]. 32 q heads / 8 kv heads (GQA group 4),
head_dim 128, causal sliding window 1024.

Sharding (tensor parallel over heads): core c owns q heads 4c..4c+3 and kv head c
(wq/wk/wv column shards). The output projection is column-sharded: each core
AllGathers the (bf16) attention outputs of all heads per 512-token chunk and
computes out[:, 512c:512c+512] with its wo column shard. Host concatenates.

All matmuls run in bf16 with f32 PSUM accumulation. Layout choices:
 - x is pre-transposed on host to x_T [4096, 2048]; per-core QKV computes
   q/k transposed [head_dim, seq] via lhsT=weight tile, rhs=x_T.
 - head_dim is pre-permuted (even indices then odd) in wq/wk columns so RoPE
   operates on contiguous partition halves [0:64], [64:128].
 - scores are computed transposed S_T[j, i], exp'd on the scalar engine
   (1/sqrt(128) folded into the activation scale), masked multiplicatively,
   and fed straight into PV with v tiles [j, d] as stationary.
 - softmax denominator: ones-vector matmul accumulated alongside PV;
   normalization = fast-approx reciprocal + gpsimd partition broadcast.
 - sliding window: per 512-query chunk only j-blocks in [i0-1024, i0+512),
   boundary blocks narrowed to their non-zero column span.
DMAs are batched (multiple 128-row tiles per transfer via 3D access patterns)
and split across the sync and gpsimd queues to keep descriptor issue off the
critical path.
"""

import math
from contextlib import ExitStack

import numpy as np

import concourse.bass as bass
import concourse.mybir as mybir
import concourse.tile as tile
from concourse import bacc
from concourse.bass_utils import run_bass_kernel_spmd
from concourse.masks import make_identity

# ---- problem constants (hardcoded; kernel.py must be self-contained) ----
B = 1
S = 2048
D = 4096
N_Q_HEADS = 32
HD = 128
WINDOW = 1024
N_CORES = 8

QH = N_Q_HEADS // N_CORES  # 4 local q heads
P = 128
CH = 512  # seq chunk
NCH = S // CH  # 4
DT = D // P  # 32 contraction tiles
ST = CH // P  # 4 s-tiles per chunk
DC = D // N_CORES  # 512 output columns per core

F32 = mybir.dt.float32
BF16 = mybir.dt.bfloat16

_BUILT = None


def _span(rel):
    """Non-zero column span (c0, c1) and mask slot for a j-block at relative
    position rel = (j0 - (i0 - WINDOW)) // 128 in 0..11."""
    if rel <= 3:
        return 0, 128 * (rel + 1), rel  # window-edge wedge
    if rel <= 7:
        return 0, CH, None  # fully inside window
    return 128 * (rel - 8), CH, 4 + (rel - 8)  # causal wedge


def _build():
    nc = bacc.Bacc("TRN2", target_bir_lowering=False, debug=False, num_devices=N_CORES)

    xT_ext = nc.declare_dram_parameter("xT", [D, S], BF16, isOutput=False)
    wq_ext = nc.declare_dram_parameter("wq", [D, QH * HD], BF16, isOutput=False)
    wk_ext = nc.declare_dram_parameter("wk", [D, HD], BF16, isOutput=False)
    wv_ext = nc.declare_dram_parameter("wv", [D, HD], BF16, isOutput=False)
    wo_ext = nc.declare_dram_parameter("wo", [D, DC], BF16, isOutput=False)
    cos_ext = nc.declare_dram_parameter("cosT", [HD // 2, S], BF16, isOutput=False)
    sin_ext = nc.declare_dram_parameter("sinT", [HD // 2, S], BF16, isOutput=False)
    mask_ext = nc.declare_dram_parameter("masks", [8, P, CH], BF16, isOutput=False)
    out_ext = nc.declare_dram_parameter("out", [S, DC], F32, isOutput=True)

    inv_sqrt_hd = 1.0 / math.sqrt(HD)

    with tile.TileContext(nc) as tc:
        with ExitStack() as stack:
            pool = lambda *a, **kw: stack.enter_context(tc.tile_pool(*a, **kw))
            wq_pool = pool(name="wq", bufs=DT // 2)  # 16 x [128, 2, 512]
            wk_pool = pool(name="wk", bufs=DT // 8)  # 4 x [128, 8, 128]
            wv_pool = pool(name="wv", bufs=DT // 8)
            x_pool = pool(name="xbf", bufs=32)  # [128, 2, 512] pairs
            k_pool = pool(name="kt", bufs=NCH)
            v_pool = pool(name="vt", bufs=12)
            q_pool = pool(name="qt", bufs=8)
            att_pool = pool(name="att", bufs=6)
            mask_pool = pool(name="mask", bufs=1)
            e_pool = pool(name="et", bufs=4)
            r_pool = pool(name="rtmp", bufs=4)
            vts_pool = pool(name="vts", bufs=1)
            rb_pool = pool(name="rb", bufs=2)
            rc_pool = pool(name="rc", bufs=2)
            at_pool = pool(name="atst", bufs=3)  # [128, 32, 128] gathered att
            wot_pool = pool(name="wot", bufs=4)  # [128, 4, 512] wo stream
            out_pool = pool(name="osb", bufs=1)
            misc_pool = pool(name="misc", bufs=1)
            acc_ps = pool(name="accps", bufs=3, space="PSUM")
            qkv_ps = acc_ps
            s_ps = acc_ps
            pv_ps = pool(name="pvps", bufs=2, space="PSUM")
            rs_ps = pool(name="rsps", bufs=1, space="PSUM")
            op_ps = pool(name="opps", bufs=2, space="PSUM")
            ccin_pool = pool(name="ccin", bufs=4, space="DRAM")
            gath_pool = pool(name="gath", bufs=4, space="DRAM")

            # ---- small constants (needed by chunk 0) ----
            cos_sb = misc_pool.tile([HD // 2, S], BF16, tag="cos")
            sin_sb = misc_pool.tile([HD // 2, S], BF16, tag="sin")
            ident = misc_pool.tile([P, P], BF16, tag="ident")
            make_identity(nc, ident[:])
            ones_bf = misc_pool.tile([P, 1], BF16, tag="ones")
            nc.vector.memset(ones_bf[:], 1.0)
            mask_all = mask_pool.tile([P, 8, CH], BF16, tag="mask", name="mask_all")
            nc.scalar.dma_start(
                out=mask_all[:], in_=mask_ext[:].rearrange("r p c -> p r c")
            )
            mask_sb = [mask_all[:, r, :] for r in range(8)]

            # tile handles
            wq_t = [None] * (DT // 2)  # [128, 2, 512] bf16
            wk_t = [None] * (DT // 8)  # [128, 8, 128] bf16
            wv_t = [None] * (DT // 8)
            x_tiles = {}  # (I, g) -> [128, 2, 512] bf16
            k_chunks = [None] * NCH
            v_tiles = [None] * (NCH * ST)
            q_tiles = {}
            att_tiles = {}
            gath = [None] * NCH
            ccin = [None] * NCH
            attn_last = {}

            def wq_ap(Dt):
                return wq_t[Dt // 2][:, Dt % 2, :]

            def x_ap(I, Dt):
                return x_tiles[(I, Dt // 2)][:, Dt % 2, :]

            def emit_x_group(I, g):
                """Load x_T rows [256g, 256(g+1)) cols of chunk I (bf16)."""
                xb = x_pool.tile([P, 2, CH], BF16, tag="xbf", name=f"xbf{I}_{g}")
                eng = (nc.sync if g % 2 == 0 else nc.scalar) if I == 0 else nc.sync
                eng.dma_start(
                    out=xb[:],
                    in_=xT_ext[
                        g * 2 * P : (g + 1) * 2 * P, I * CH : (I + 1) * CH
                    ].rearrange("(po pi) s -> pi po s", pi=P),
                )
                x_tiles[(I, g)] = xb

            def emit_x_chunk(I):
                for g in range(DT // 2):
                    emit_x_group(I, g)

            def rope(ps, out_bf, I):
                cs = cos_sb[:, I * CH : (I + 1) * CH]
                sn = sin_sb[:, I * CH : (I + 1) * CH]
                m1 = r_pool.tile([HD // 2, CH], F32, tag="m", name="m1")
                nc.vector.tensor_mul(m1[:], ps[0:64, :], cs)
                m2 = r_pool.tile([HD // 2, CH], F32, tag="m", name="m2")
                nc.vector.tensor_mul(m2[:], ps[64:128, :], sn)
                nc.vector.tensor_sub(out_bf[0:64, :], m1[:], m2[:])
                m3 = r_pool.tile([HD // 2, CH], F32, tag="m", name="m3")
                nc.vector.tensor_mul(m3[:], ps[0:64, :], sn)
                m4 = r_pool.tile([HD // 2, CH], F32, tag="m", name="m4")
                nc.vector.tensor_mul(m4[:], ps[64:128, :], cs)
                nc.vector.tensor_add(out_bf[64:128, :], m3[:], m4[:])

            def emit_qkv(I):
                for h in range(QH):
                    ps = qkv_ps.tile([P, CH], F32, tag="acc", name=f"psq{I}_{h}")
                    for Dt in range(DT):
                        nc.tensor.matmul(
                            ps[:],
                            wq_ap(Dt)[:, h * HD : (h + 1) * HD],
                            x_ap(I, Dt),
                            start=(Dt == 0),
                            stop=(Dt == DT - 1),
                        )
                    qb = q_pool.tile([P, CH], BF16, tag="qb", name=f"qb{I}_{h}")
                    rope(ps, qb, I)
                    q_tiles[(I, h)] = qb
                psk = qkv_ps.tile([P, CH], F32, tag="acc", name=f"psk{I}")
                for Dt in range(DT):
                    nc.tensor.matmul(
                        psk[:],
                        wk_t[Dt // 8][:, Dt % 8, :],
                        x_ap(I, Dt),
                        start=(Dt == 0),
                        stop=(Dt == DT - 1),
                    )
                kb = k_pool.tile([P, CH], BF16, tag="kb", name=f"kb{I}")
                rope(psk, kb, I)
                k_chunks[I] = kb
                psv = qkv_ps.tile([P, CH], F32, tag="acc", name=f"psv{I}")
                for Dt in range(DT):
                    nc.tensor.matmul(
                        psv[:],
                        wv_t[Dt // 8][:, Dt % 8, :],
                        x_ap(I, Dt),
                        start=(Dt == 0),
                        stop=(Dt == DT - 1),
                    )
                vT = vts_pool.tile([P, CH], BF16, tag="vT", name=f"vT{I}")
                nc.vector.tensor_copy(vT[:], psv[:])
                for sb in range(ST):
                    trp = qkv_ps.tile([P, P], BF16, tag="acc", name=f"trp{I}_{sb}")
                    nc.tensor.transpose(trp[:], vT[:, sb * P : (sb + 1) * P], ident[:])
                    vb = v_pool.tile([P, P], BF16, tag="vb", name=f"vb{I}_{sb}")
                    nc.vector.tensor_copy(vb[:], trp[:])
                    v_tiles[I * ST + sb] = vb

            def emit_attn(I):
                ccin[I] = ccin_pool.tile(
                    [QH * HD, CH], BF16, tag="ci", name=f"cin{I}"
                )
                i0 = I * CH
                jlo = max(0, i0 - WINDOW)
                n_j = (i0 + CH - jlo) // P
                for h in range(QH):
                    pv = pv_ps.tile([P, CH], F32, tag="pv", name=f"pv{I}_{h}")
                    rs = rs_ps.tile([1, CH], F32, tag="rs", name=f"rs{I}_{h}")
                    qb = q_tiles[(I, h)]
                    for idx in range(n_j):
                        j0 = jlo + idx * P
                        rel = (j0 - (i0 - WINDOW)) // P
                        c0, c1, slot = _span(rel)
                        kb = k_chunks[j0 // CH]
                        koff = j0 % CH
                        sps = s_ps.tile([P, CH], F32, tag="acc", name=f"sps{I}_{h}_{idx}")
                        nc.tensor.matmul(
                            sps[:, c0:c1],
                            kb[:, koff : koff + P],
                            qb[:, c0:c1],
                            start=True,
                            stop=True,
                            skip_group_check=True,
                        )
                        et = e_pool.tile([P, CH], BF16, tag="et", name=f"et{I}_{h}_{idx}")
                        nc.scalar.activation(
                            et[:, c0:c1],
                            sps[:, c0:c1],
                            mybir.ActivationFunctionType.Exp,
                            scale=inv_sqrt_hd,
                        )
                        if slot is not None:
                            nc.vector.tensor_mul(
                                et[:, c0:c1], et[:, c0:c1], mask_sb[slot][:, c0:c1]
                            )
                        nc.tensor.matmul(
                            pv[:, c0:c1],
                            v_tiles[j0 // P][:],
                            et[:, c0:c1],
                            start=(idx == 0),
                            stop=(idx == n_j - 1),
                            skip_group_check=True,
                        )
                        rs_mm = nc.tensor.matmul(
                            rs[:, c0:c1],
                            ones_bf[:],
                            et[:, c0:c1],
                            start=(idx == 0),
                            stop=(idx == n_j - 1),
                            skip_group_check=True,
                        )
                        attn_last[I] = rs_mm
                    # copy the rowsum out of PSUM first (~0.5us) so the rs
                    # bank frees for the next head before the slow reciprocal
                    rss = rc_pool.tile([1, CH], F32, tag="rss", name=f"rss{I}_{h}")
                    nc.vector.tensor_copy(rss[:], rs[:])
                    rc = rc_pool.tile([1, CH], F32, tag="rc", name=f"rc{I}_{h}")
                    rb = rb_pool.tile([P, CH], F32, tag="rb", name=f"rb{I}_{h}")
                    ab = att_pool.tile([P, CH], BF16, tag="ab", name=f"ab{I}_{h}")
                    for ci_ in range(2):
                        sl = slice(ci_ * (CH // 2), (ci_ + 1) * (CH // 2))
                        nc.vector.reciprocal(rc[:, sl], rss[:, sl])
                        nc.gpsimd.partition_broadcast(rb[:, sl], rc[:, sl])
                        nc.vector.tensor_mul(ab[:, sl], pv[:, sl], rb[:, sl])
                    att_tiles[(I, h)] = ab
                    emit_ship_head(I, h)
                if True:
                    emit_ag(I)

            def emit_ship_head(I, h):
                nc.gpsimd.dma_start(
                    out=ccin[I][h * HD : (h + 1) * HD, :], in_=att_tiles[(I, h)][:]
                )

            def emit_ag(I):
                go = gath_pool.tile(
                    [D, CH], BF16, addr_space="Shared", tag="go", name=f"go{I}"
                )
                nc.gpsimd.collective_compute(
                    "AllGather",
                    mybir.AluOpType.bypass,
                    replica_groups=[list(range(N_CORES))],
                    ins=[ccin[I][:].opt()],
                    outs=[go[:].opt()],
                )
                gath[I] = go

            def emit_outproj(I):
                for g in range(ST // 2):
                    pso = [
                        op_ps.tile([P, CH], F32, tag="op", name=f"pso{I}_{g}_{k}")
                        for k in range(2)
                    ]
                    ats = {}
                    for half in range(2):
                        for k in range(2):
                            st_idx = g * 2 + k
                            at = at_pool.tile(
                                [P, DT // 2, P],
                                BF16,
                                tag="at",
                                name=f"at{I}_{st_idx}_{half}",
                            )
                            nc.scalar.dma_start(
                                out=at[:],
                                in_=gath[I][
                                    half * (D // 2) : (half + 1) * (D // 2),
                                    st_idx * P : (st_idx + 1) * P,
                                ].rearrange("(po pi) c -> pi po c", pi=P),
                            )
                            ats[(half, k)] = at
                    for Dtg in range(DT // 4):
                        wt = wot_pool.tile(
                            [P, 4, CH], BF16, tag="wt", name=f"wt{I}_{g}_{Dtg}"
                        )
                        weng = nc.sync if Dtg % 2 == 0 else nc.gpsimd
                        weng.dma_start(
                            out=wt[:],
                            in_=wo_ext[Dtg * 4 * P : (Dtg + 1) * 4 * P, :].rearrange(
                                "(po pi) e -> pi po e", pi=P
                            ),
                        )
                        for d4 in range(4):
                            Dt = Dtg * 4 + d4
                            for k in range(2):
                                op_mm = nc.tensor.matmul(
                                    pso[k][:],
                                    ats[(Dt // (DT // 2), k)][:, Dt % (DT // 2), :],
                                    wt[:, d4, :],
                                    start=(Dt == 0),
                                    stop=(Dt == DT - 1),
                                )
                                if g == 0 and Dt == 0 and k == 0 and (I + 1) in attn_last:
                                    bass._add_dep_helper(
                                        op_mm.ins,
                                        attn_last[I + 1].ins,
                                        sync=False,
                                        reason="outproj after next attn",
                                    )
                    for k in range(2):
                        st_idx = g * 2 + k
                        ob = out_pool.tile([P, CH], F32, tag="ob", name=f"ob{I}_{st_idx}")
                        nc.vector.tensor_copy(ob[:], pso[k][:])
                        nc.gpsimd.dma_start(
                            out=out_ext[
                                I * CH + st_idx * P : I * CH + (st_idx + 1) * P, :
                            ],
                            in_=ob[:],
                        )

            # ---- emission schedule ----
            # chunk 0: interleave x groups with weight groups so QKV starts early
            for g in range(DT // 2):
                emit_x_group(0, g)
                wqb = wq_pool.tile([P, 2, QH * HD], BF16, tag="wqb", name=f"wqb{g}")
                nc.gpsimd.dma_start(
                    out=wqb[:],
                    in_=wq_ext[g * 2 * P : (g + 1) * 2 * P, :].rearrange(
                        "(po pi) c -> pi po c", pi=P
                    ),
                )
                wq_t[g] = wqb
                if g == 3:
                    nc.scalar.dma_start(out=cos_sb[:], in_=cos_ext[:])
                    nc.scalar.dma_start(out=sin_sb[:], in_=sin_ext[:])
                if g % 4 == 0:
                    gg = g // 4
                    wkb = wk_pool.tile([P, 8, HD], BF16, tag="wkb", name=f"wkb{gg}")
                    nc.sync.dma_start(
                        out=wkb[:],
                        in_=wk_ext[gg * 8 * P : (gg + 1) * 8 * P, :].rearrange(
                            "(po pi) c -> pi po c", pi=P
                        ),
                    )
                    wk_t[gg] = wkb
                    wvb = wv_pool.tile([P, 8, HD], BF16, tag="wvb", name=f"wvb{gg}")
                    nc.scalar.dma_start(
                        out=wvb[:],
                        in_=wv_ext[gg * 8 * P : (gg + 1) * 8 * P, :].rearrange(
                            "(po pi) c -> pi po c", pi=P
                        ),
                    )
                    wv_t[gg] = wvb

            emit_qkv(0)
            emit_x_chunk(1)
            emit_attn(0)

            emit_qkv(1)
            emit_x_chunk(2)
            emit_attn(1)
            emit_outproj(0)

            emit_qkv(2)
            emit_x_chunk(3)
            emit_attn(2)
            emit_outproj(1)

            emit_qkv(3)
            emit_attn(3)
            emit_outproj(2)
            emit_outproj(3)

    nc.compile()
    return nc


def _prep_inputs(x, freqs_cos, freqs_sin, wq, wk, wv, wo):
    """Shard + lay out the full inputs for the 8 cores."""
    xT = np.ascontiguousarray(x.reshape(S, D).T).astype(np.float32)
    cosT = np.ascontiguousarray(freqs_cos.T).astype(np.float32)
    sinT = np.ascontiguousarray(freqs_sin.T).astype(np.float32)

    perm = np.concatenate([np.arange(0, HD, 2), np.arange(1, HD, 2)])

    import ml_dtypes

    jj = np.arange(P)[:, None]
    ii = np.arange(CH)[None, :]
    masks = np.zeros((8, P, CH), ml_dtypes.bfloat16)
    for r in range(4):
        masks[r] = (ii - jj <= 128 * r).astype(ml_dtypes.bfloat16)
    for r in range(8, 12):
        masks[4 + r - 8] = (ii - jj >= 128 * (r - 8)).astype(ml_dtypes.bfloat16)

    bf = ml_dtypes.bfloat16
    xT_bf = xT.astype(bf)
    cosT_bf = cosT.astype(bf)
    sinT_bf = sinT.astype(bf)
    in_maps = []
    for c in range(N_CORES):
        q_cols = np.concatenate([(QH * c + h) * HD + perm for h in range(QH)])
        k_cols = c * HD + perm
        in_maps.append(
            {
                "xT": xT_bf,
                "wq": np.ascontiguousarray(wq[:, q_cols]).astype(bf),
                "wk": np.ascontiguousarray(wk[:, k_cols]).astype(bf),
                "wv": np.ascontiguousarray(wv[:, c * HD : (c + 1) * HD]).astype(bf),
                "wo": np.ascontiguousarray(wo[:, c * DC : (c + 1) * DC]).astype(bf),
                "cosT": cosT_bf,
                "sinT": sinT_bf,
                "masks": masks,
            }
        )
    return in_maps


def kernel(x, freqs_cos, freqs_sin, wq, wk, wv, wo, _trace=False, _result_box=None):
    global _BUILT
    x = np.asarray(x, dtype=np.float32)
    if _BUILT is None:
        _BUILT = _build()
    nc = _BUILT
    in_maps = _prep_inputs(
        x,
        np.asarray(freqs_cos, np.float32),
        np.asarray(freqs_sin, np.float32),
        np.asarray(wq, np.float32),
        np.asarray(wk, np.float32),
        np.asarray(wv, np.float32),
        np.asarray(wo, np.float32),
    )
    res = run_bass_kernel_spmd(nc, in_maps, core_ids=list(range(N_CORES)), trace=_trace)
    if _result_box is not None:
        _result_box.append(res)
    out = np.concatenate([res.results[c]["out"] for c in range(N_CORES)], axis=1)
    return out.reshape(B, S, D).astype(np.float32)
